# revision 1
# baseline (speedup 1.0000x reference)
"""MoE dispatch/combine kernel for Trainium2 (8 NeuronCores, token-parallel).

Computes, for hidden_states [B=4, S=4096, H=2048], router_weight [E=64, H],
router_bias [E], expert_bias [E, H], TOP_K=8:

    logits = x @ rw.T + rb ; scores = softmax(logits) ; top8
    out = x * (sum top8 scores) + (top8-masked scores) @ expert_bias

Per core (2048 tokens, no collectives -- pure token parallelism):
  - x tiles [128, 2048] stream in as f32r; a fp16 shadow (DVE cast) feeds the
    router path.
  - PE transposes the fp16 shadow chunkwise (transpose-mode, 16-bit = fast
    weight loads); router matmul runs in fp16 against pre-transposed router
    weights, accumulating logitsT [64, blk] fp32 in PSUM; router bias is
    added during the PSUM->SBUF copy (per-partition bias = per-expert).
  - Logits transpose back to [token, expert] (fp32); DVE Max8 yields the
    top-8 values per token in one op; ACT exp with per-token bias and
    accumulated row sums gives softmax pieces; one chained tensor_scalar
    builds the top-8 mask * 1/Z; C = masked normalized scores (fp16).
  - Output = C @ expert_bias (fp16) + diag(a) @ x (f32r, keeps the dominant
    x*a term at 12-bit precision), accumulated in one PSUM bank; ACT/DVE
    copy out; DMA back.

fp16 router path: logits have std ~45 so softmax is extremely peaked;
fp16's 10-bit mantissa gives logit noise ~0.03, which only matters for
near-tied experts whose scores are nearly equal anyway (measured end-to-end
~3e-4 relative error).
"""
import os
import sys

for _p in ("/opt/trn_rl_repo", "/opt/pypackages"):
    if _p not in sys.path:
        sys.path.append(_p)

os.environ.setdefault("BASS_NEVER_TRACE", "1")

import numpy as np
from contextlib import ExitStack

import concourse.bass as bass
import concourse.tile as tile
from concourse import bacc, mybir
from concourse.bass_utils import run_bass_kernel_spmd

F32 = mybir.dt.float32
F32R = mybir.dt.float32r
F16 = mybir.dt.float16
AF = mybir.ActivationFunctionType
AL = mybir.AluOpType

B, S, H, E, TOPK = 4, 4096, 2048, 64, 8
T = B * S
N_CORES = 8
T_PC = T // N_CORES            # 2048 tokens per core
BLK = 256                      # tokens per block
N_BLK = T_PC // BLK            # 8
TPB = BLK // 128               # 2 tiles per block
HCH = H // 128                 # 16 h-chunks
OW = 512                       # output PSUM bank width (fp32)


def _build():
    nc = bacc.Bacc("TRN2", target_bir_lowering=False, debug=False,
                   num_devices=N_CORES)

    x_d = nc.dram_tensor("x", [T_PC, H], F32R, kind="ExternalInput").ap()
    # pre-arranged on host: [h-in-chunk, chunk, expert] so the DMA is contiguous
    rwt_d = nc.dram_tensor("rwt", [128, HCH * E], F16, kind="ExternalInput").ap()
    eb_d = nc.dram_tensor("eb", [E, H], F16, kind="ExternalInput").ap()
    rb_d = nc.dram_tensor("rb", [E, 1], F32, kind="ExternalInput").ap()
    idr_d = nc.dram_tensor("idr", [128, 128], F32R, kind="ExternalInput").ap()
    idf_d = nc.dram_tensor("idf", [128, 128], F32, kind="ExternalInput").ap()
    idh_d = nc.dram_tensor("idh", [128, 128], F16, kind="ExternalInput").ap()
    out_d = nc.dram_tensor("out", [T_PC, H], F16, kind="ExternalOutput").ap()

    with tile.TileContext(nc) as tc:
        with ExitStack() as ctx:
            consts = ctx.enter_context(tc.tile_pool(name="consts", bufs=1))
            xp = ctx.enter_context(tc.tile_pool(name="xp", bufs=6))
            xbp = ctx.enter_context(tc.tile_pool(name="xbp", bufs=4))
            xtp = ctx.enter_context(tc.tile_pool(name="xtp", bufs=3))
            lgp = ctx.enter_context(tc.tile_pool(name="lgp", bufs=2))
            wp = ctx.enter_context(tc.tile_pool(name="wp", bufs=2))
            yp = ctx.enter_context(tc.tile_pool(name="yp", bufs=2))
            stp = ctx.enter_context(tc.tile_pool(name="stp", bufs=2))
            cp = ctx.enter_context(tc.tile_pool(name="cp", bufs=4))
            op = ctx.enter_context(tc.tile_pool(name="op", bufs=3))

            tp_ps = ctx.enter_context(
                tc.tile_pool(name="tp_ps", bufs=2, space="PSUM"))
            lg_ps = ctx.enter_context(
                tc.tile_pool(name="lg_ps", bufs=2, space="PSUM"))
            w_ps = ctx.enter_context(
                tc.tile_pool(name="w_ps", bufs=1, space="PSUM"))
            ct_ps = ctx.enter_context(
                tc.tile_pool(name="ct_ps", bufs=1, space="PSUM"))
            out_ps = ctx.enter_context(
                tc.tile_pool(name="out_ps", bufs=2, space="PSUM"))

            # ---- constants (identity first: transposes need it on tick 1) ----
            idh = consts.tile([128, 128], F16)
            nc.sync.dma_start(idh[:], idh_d)
            idr = consts.tile([128, 128], F32R)
            nc.sync.dma_start(idr[:], idr_d)
            idf = consts.tile([128, 128], F32)
            nc.sync.dma_start(idf[:], idf_d)
            rb = consts.tile([E, 1], F32)
            nc.sync.dma_start(rb[:], rb_d)
            rwt = consts.tile([128, HCH, E], F16)
            nc.sync.dma_start(rwt[:].rearrange("p c e -> p (c e)"), rwt_d)
            eb = consts.tile([E, H], F16)
            nc.sync.dma_start(eb[:], eb_d)

            for b in range(N_BLK):
                t0 = b * BLK
                xs = []
                xT = xtp.tile([128, HCH, BLK], F16)
                for j in range(TPB):
                    xt = xp.tile([128, H], F32R, tag=f"x{j}")
                    nc.sync.dma_start(xt[:], x_d[t0 + 128 * j:t0 + 128 * (j + 1), :])
                    xs.append(xt)
                    xb = xbp.tile([128, H], F16, tag=f"xb{j}")
                    # split casts across engines: the two per-block casts gate
                    # the transpose chain and serialize on the DVE otherwise
                    if j % 2 == 0:
                        nc.vector.tensor_copy(xb[:], xt[:].bitcast(F32))
                    else:
                        nc.scalar.copy(xb[:], xt[:].bitcast(F32))
                    for c0 in range(0, HCH, 8):
                        tp = tp_ps.tile([128, 1024], F16, tag="tp")
                        for ci in range(8):
                            c = c0 + ci
                            nc.tensor.matmul(
                                tp[:, 128 * ci:128 * (ci + 1)],
                                xb[:, 128 * c:128 * (c + 1)],
                                idh[:], is_transpose=True,
                                start=(ci == 0), stop=(ci == 7))
                        nc.any.tensor_copy(
                            xT[:, c0:c0 + 8, 128 * j:128 * (j + 1)], tp[:])

                # ---- router matmul (fp16): logitsT [E, BLK] ----
                lg = lg_ps.tile([E, BLK], F32, tag="lg")
                for c in range(HCH):
                    nc.tensor.matmul(lg[:], rwt[:, c, :], xT[:, c, :],
                                     start=(c == 0), stop=(c == HCH - 1))
                lgs = lgp.tile([E, BLK], F32)
                nc.scalar.activation(lgs[:], lg[:], AF.Identity,
                                     bias=rb[:], scale=1.0)

                # ---- logits back to [token, expert] ----
                wps = w_ps.tile([128, TPB * E], F32, tag="wps")
                for j in range(TPB):
                    nc.tensor.matmul(
                        wps[:, E * j:E * (j + 1)],
                        lgs[:, 128 * j:128 * (j + 1)],
                        idf[0:E, 0:E], is_transpose=True,
                        start=(j == 0), stop=(j == TPB - 1))
                w = wp.tile([128, TPB, E], F32)
                nc.any.tensor_copy(w[:], wps[:])

                # ---- softmax + top8 stats ----
                top8 = stp.tile([128, TPB, TOPK], F32, tag="top8")
                for j in range(TPB):
                    nc.vector.max(top8[:, j, :], w[:, j, :])
                negm = stp.tile([128, TPB], F32, tag="negm")
                nc.vector.tensor_scalar(negm[:], top8[:, :, 0], -1.0, None, AL.mult)
                y = yp.tile([128, TPB, E], F32)
                z = stp.tile([128, TPB], F32, tag="z")
                e8 = stp.tile([128, TPB, TOPK], F32, tag="e8")
                s8 = stp.tile([128, TPB], F32, tag="s8")
                for j in range(TPB):
                    nc.scalar.activation(y[:, j, :], w[:, j, :], AF.Exp,
                                         bias=negm[:, j:j + 1], scale=1.0,
                                         accum_out=z[:, j:j + 1])
                    nc.scalar.activation(e8[:, j, :], top8[:, j, :], AF.Exp,
                                         bias=negm[:, j:j + 1], scale=1.0,
                                         accum_out=s8[:, j:j + 1])
                iz = stp.tile([128, TPB], F32, tag="iz")
                nc.vector.reciprocal(iz[:], z[:])
                a = stp.tile([128, TPB], F32, tag="a")
                nc.vector.tensor_tensor(a[:], s8[:], iz[:], op=AL.mult)

                # ---- per block: masks + stationaries for both tiles ----
                ctss, diags = [], []
                for j in range(TPB):
                    g = cp.tile([128, E], F32, tag=f"g{j}")
                    nc.vector.tensor_scalar(g[:], w[:, j, :],
                                            top8[:, j, TOPK - 1:TOPK],
                                            iz[:, j:j + 1], AL.is_ge, AL.mult)
                    c_t = cp.tile([128, E], F16, tag=f"c{j}")
                    nc.vector.tensor_tensor(c_t[:], y[:, j, :], g[:], op=AL.mult)
                    ct = ct_ps.tile([E, 128], F16, tag="ct")
                    nc.tensor.matmul(ct[:], c_t[:], idh[:], is_transpose=True,
                                     start=True, stop=True)
                    cts = cp.tile([E, 128], F16, tag=f"cts{j}")
                    nc.any.tensor_copy(cts[:], ct[:])
                    ctss.append(cts)
                    diag = cp.tile([128, 128], F32R, tag=f"diag{j}")
                    nc.vector.tensor_scalar(diag[:], idr[:], a[:, j:j + 1],
                                            None, AL.mult)
                    diags.append(diag)

                # ---- clustered output matmuls (keep the PE burst dense) ----
                os_ = [op.tile([128, H], F16, tag=f"o{j}", name=f"o{j}_{b}")
                       for j in range(TPB)]
                pss = []
                for j in range(TPB):
                    for k in range(H // OW):
                        ops_ = out_ps.tile([128, OW], F32, tag="ops")
                        nc.tensor.matmul(ops_[:], ctss[j][:],
                                         eb[:, OW * k:OW * (k + 1)],
                                         start=True, stop=False)
                        nc.tensor.matmul(ops_[:], diags[j][:],
                                         xs[j][:, OW * k:OW * (k + 1)],
                                         start=False, stop=True)
                        nc.any.tensor_copy(os_[j][:, OW * k:OW * (k + 1)], ops_[:])
                for j in range(TPB):
                    # separate HWDGE ring: out-DMAs must not queue behind the
                    # deep x-prefetch FIFO on the sync ring
                    nc.sync.dma_start(
                        out_d[t0 + 128 * j:t0 + 128 * (j + 1), :], os_[j][:])

    nc.compile()
    return nc


_NC_CACHE = None


def _get_nc():
    global _NC_CACHE
    if _NC_CACHE is None:
        _NC_CACHE = _build()
    return _NC_CACHE


def _prep_inputs(hidden_states, router_weight, router_bias, expert_bias):
    import ml_dtypes  # noqa: F401
    flat = np.ascontiguousarray(hidden_states.reshape(T, H), dtype=np.float32)
    # [H, E] -> [h-in-chunk(128), chunk(16)*expert(64)] contiguous
    rwt = np.ascontiguousarray(
        router_weight.T.reshape(HCH, 128, E).transpose(1, 0, 2).reshape(128, HCH * E)
    ).astype(np.float16)
    rb = np.ascontiguousarray(router_bias.reshape(E, 1)).astype(np.float32)
    eb = np.ascontiguousarray(expert_bias).astype(np.float16)
    eye = np.eye(128, dtype=np.float32)
    eye_h = eye.astype(np.float16)
    in_maps = []
    for c in range(N_CORES):
        in_maps.append({
            "x": flat[c * T_PC:(c + 1) * T_PC],
            "rwt": rwt,
            "eb": eb,
            "rb": rb,
            "idr": eye,
            "idf": eye,
            "idh": eye_h,
        })
    return in_maps


def kernel(hidden_states, router_weight, router_bias, expert_bias):
    hidden_states = np.asarray(hidden_states, dtype=np.float32)
    router_weight = np.asarray(router_weight, dtype=np.float32)
    router_bias = np.asarray(router_bias, dtype=np.float32)
    expert_bias = np.asarray(expert_bias, dtype=np.float32)
    assert hidden_states.shape == (B, S, H)

    nc = _get_nc()
    in_maps = _prep_inputs(hidden_states, router_weight, router_bias, expert_bias)
    res = run_bass_kernel_spmd(nc, in_maps, list(range(N_CORES)))
    out = np.concatenate([res.results[c]["out"] for c in range(N_CORES)], axis=0)
    return out.astype(np.float32).reshape(B, S, H)


if __name__ == "__main__":
    rng = np.random.default_rng(0)
    hs = rng.standard_normal((B, S, H), dtype=np.float32)
    rw = rng.standard_normal((E, H), dtype=np.float32)
    rbv = np.zeros((E,), dtype=np.float32)
    ebv = (rng.standard_normal((E, H), dtype=np.float32) * 0.1).astype(np.float32)
    o = kernel(hidden_states=hs, router_weight=rw, router_bias=rbv, expert_bias=ebv)
    print("kernel out", o.shape, o.dtype, float(np.abs(o).mean()))



# revision 2
# speedup vs baseline: 1.2121x; 1.2121x over previous
"""MoE dispatch/combine kernel for Trainium2 (8 NeuronCores, token-parallel).

Computes, for hidden_states [B=4, S=4096, H=2048], router_weight [E=64, H],
router_bias [E], expert_bias [E, H], TOP_K=8:

    logits = x @ rw.T + rb ; scores = softmax(logits) ; top8
    out = x * (sum top8 scores) + (top8-masked scores) @ expert_bias

Fully *transposed* dataflow (per core: 2048 tokens, no collectives):
  - Host supplies x already transposed + fp16: xt[p, g, c, i] = x[512g+i, 128c+p]
    (g: 4 groups of 512 tokens, c: 16 h-chunks). One 8.4MB input copy; 8.4MB
    fp16 transposed output; host undoes the transpose. Total HBM traffic per
    core ~17MB vs ~26MB for the untransposed design -- and the PE never
    transposes x (the old kernel spent ~40% of PE time on x transposes).
  - Router: logitsT[64, t] accumulates rw_chunk.T @ xt_chunk (rw stationary).
  - Small PE transposes move logits to [t, e] for the DVE Max8 top-8 path;
    masked normalized scores c[t, e] (fp16) transpose back to cT[e, t].
  - a[t] = sum_e c[t, e] materializes *broadcast across partitions* with one
    ones[64,128].T @ cT matmul -- no per-token reduction op needed.
  - Combine: outT[h-chunk, t] = eb_chunk.T @ cT (eb is a natural lhsT in
    [E, H] layout -- no transpose), drained by DVE as psum + a*xt in one
    tensor_tensor over a precomputed axt = xt * a_bcast tile.

fp16 x/router: logits have std ~45 so softmax is extremely peaked; fp16
logit noise ~0.03 only reorders near-tied experts (harmless). fp16 x on the
dominant x*a term gives ~3e-4 relative error (gate is 2e-2).
"""
import os
import sys

for _p in ("/opt/trn_rl_repo", "/opt/pypackages"):
    if _p not in sys.path:
        sys.path.append(_p)

os.environ.setdefault("BASS_NEVER_TRACE", "1")

import numpy as np
from contextlib import ExitStack

import concourse.bass as bass
import concourse.tile as tile
from concourse import bacc, mybir
from concourse.bass_utils import run_bass_kernel_spmd

F32 = mybir.dt.float32
F16 = mybir.dt.float16
AF = mybir.ActivationFunctionType
AL = mybir.AluOpType

B, S, H, E, TOPK = 4, 4096, 2048, 64, 8
T = B * S
N_CORES = 8
T_PC = T // N_CORES            # 2048 tokens per core
NG = 4                         # token groups per core
GT = T_PC // NG                # 512 tokens per group
NTIL = GT // 128               # 4 token tiles per group
HCH = H // 128                 # 16 h-chunks
CPB = 2                        # h-chunks per output psum tile (2 banks)


def _build():
    nc = bacc.Bacc("TRN2", target_bir_lowering=False, debug=False,
                   num_devices=N_CORES)

    # xt[p, g, c, i] = x[t=512g+i, h=128c+p], fp16, flat [128, NG*HCH*GT]
    xt_d = nc.dram_tensor("xt", [128, NG * HCH * GT], F16,
                          kind="ExternalInput").ap()
    # rwt[p, c*E+e] = rw[e, 128c+p]
    rwt_d = nc.dram_tensor("rwt", [128, HCH * E], F16, kind="ExternalInput").ap()
    eb_d = nc.dram_tensor("eb", [E, H], F16, kind="ExternalInput").ap()
    rb_d = nc.dram_tensor("rb", [E, 1], F32, kind="ExternalInput").ap()
    idf_d = nc.dram_tensor("idf", [128, 128], F32, kind="ExternalInput").ap()
    idh_d = nc.dram_tensor("idh", [128, 128], F16, kind="ExternalInput").ap()
    ones_d = nc.dram_tensor("ones", [E, 128], F16, kind="ExternalInput").ap()
    # out[p, g, c, i] = out[t=512g+i, h=128c+p], fp16
    out_d = nc.dram_tensor("out", [128, NG * HCH * GT], F16,
                           kind="ExternalOutput").ap()

    with tile.TileContext(nc) as tc:
        with ExitStack() as ctx:
            consts = ctx.enter_context(tc.tile_pool(name="consts", bufs=1))
            lgsp = ctx.enter_context(tc.tile_pool(name="lgsp", bufs=2))
            wsb = ctx.enter_context(tc.tile_pool(name="wsb", bufs=2))
            stp = ctx.enter_context(tc.tile_pool(name="stp", bufs=2))
            ctp = ctx.enter_context(tc.tile_pool(name="ctp", bufs=2))
            abp = ctx.enter_context(tc.tile_pool(name="abp", bufs=2))
            axp = ctx.enter_context(tc.tile_pool(name="axp", bufs=2))
            osb = ctx.enter_context(tc.tile_pool(name="osb", bufs=4))

            lg_ps = ctx.enter_context(
                tc.tile_pool(name="lg_ps", bufs=1, space="PSUM"))
            w_ps = ctx.enter_context(
                tc.tile_pool(name="w_ps", bufs=1, space="PSUM"))
            ct_ps = ctx.enter_context(
                tc.tile_pool(name="ct_ps", bufs=1, space="PSUM"))
            ab_ps = ctx.enter_context(
                tc.tile_pool(name="ab_ps", bufs=1, space="PSUM"))
            out_ps = ctx.enter_context(
                tc.tile_pool(name="out_ps", bufs=2, space="PSUM"))

            # ---- constants ----
            idh = consts.tile([128, 128], F16)
            nc.sync.dma_start(idh[:], idh_d)
            idf = consts.tile([128, 128], F32)
            nc.sync.dma_start(idf[:], idf_d)
            rb = consts.tile([E, 1], F32)
            nc.sync.dma_start(rb[:], rb_d)
            ones = consts.tile([E, 128], F16)
            nc.sync.dma_start(ones[:], ones_d)
            rwt = consts.tile([128, HCH, E], F16)
            nc.sync.dma_start(rwt[:].rearrange("p c e -> p (c e)"), rwt_d)
            eb = consts.tile([E, H], F16)
            nc.sync.dma_start(eb[:], eb_d)

            # x prefetch: all 4 group slabs up front (2.1MB each, contiguous)
            xt = consts.tile([128, NG, HCH, GT], F16)
            for g in range(NG):
                nc.sync.dma_start(
                    xt[:, g].rearrange("p c i -> p (c i)"),
                    xt_d[:, g * HCH * GT:(g + 1) * HCH * GT])

            for g in range(NG):
                # ---- router: logitsT [E, GT] ----
                lg = lg_ps.tile([E, GT], F32, tag="lg")
                for c in range(HCH):
                    nc.tensor.matmul(lg[:], rwt[:, c, :], xt[:, g, c, :],
                                     start=(c == 0), stop=(c == HCH - 1))
                lgs = lgsp.tile([E, GT], F32, tag="lgs")
                nc.scalar.activation(lgs[:], lg[:], AF.Identity,
                                     bias=rb[:], scale=1.0)

                # ---- logits to [token, expert] (4 tiles of 128 tokens) ----
                wps = w_ps.tile([128, NTIL, E], F32, tag="wps")
                for i in range(NTIL):
                    nc.tensor.matmul(
                        wps[:, i, :], lgs[:, 128 * i:128 * (i + 1)],
                        idf[0:E, 0:E], is_transpose=True,
                        start=True, stop=True)
                w = wsb.tile([128, NTIL, E], F32, tag="w")
                nc.scalar.copy(w[:], wps[:])

                # ---- softmax + top8 -> masked normalized scores cT ----
                ctps = ct_ps.tile([E, NTIL, 128], F16, tag="ctps")
                for i in range(NTIL):
                    top8 = stp.tile([128, TOPK], F32, tag=f"top8_{i}")
                    nc.vector.max(top8[:], w[:, i, :])
                    negm = stp.tile([128, 1], F32, tag=f"negm_{i}")
                    nc.vector.tensor_scalar(negm[:], top8[:, 0:1], -1.0, None,
                                            AL.mult)
                    y = stp.tile([128, E], F32, tag=f"y_{i}")
                    z = stp.tile([128, 1], F32, tag=f"z_{i}")
                    nc.scalar.activation(y[:], w[:, i, :], AF.Exp,
                                         bias=negm[:], scale=1.0,
                                         accum_out=z[:])
                    iz = stp.tile([128, 1], F32, tag=f"iz_{i}")
                    nc.vector.reciprocal(iz[:], z[:])
                    g01 = stp.tile([128, E], F32, tag=f"g01_{i}")
                    nc.vector.tensor_scalar(g01[:], w[:, i, :],
                                            top8[:, TOPK - 1:TOPK],
                                            iz[:], AL.is_ge, AL.mult)
                    cmask = stp.tile([128, E], F16, tag=f"c_{i}")
                    nc.vector.tensor_tensor(cmask[:], y[:], g01[:], op=AL.mult)
                    nc.tensor.matmul(ctps[:, i, :], cmask[:], idh[:],
                                     is_transpose=True, start=True, stop=True)
                cT = ctp.tile([E, NTIL * 128], F16, tag="cT")
                nc.vector.tensor_copy(cT[:], ctps[:].rearrange("e n p -> e (n p)"))

                # ---- a[t] broadcast across partitions: ones.T @ cT ----
                abps = ab_ps.tile([128, GT], F32, tag="abps")
                nc.tensor.matmul(abps[:], ones[:], cT[:], start=True, stop=True)
                ab = abp.tile([128, GT], F16, tag="ab")
                nc.vector.tensor_copy(ab[:], abps[:])

                # ---- axt = xt * a (one big DVE op, a broadcast over chunks) ----
                axt = axp.tile([128, HCH, GT], F16, tag="axt")
                ab_bc = ab[:].unsqueeze(1).broadcast_to((128, HCH, GT))
                nc.vector.tensor_tensor(axt[:], xt[:, g], ab_bc, op=AL.mult)

                # ---- combine: outT[h, t] = eb_c.T @ cT + axt ----
                for c0 in range(0, HCH, CPB):
                    ops_ = out_ps.tile([128, CPB, GT], F32, tag="ops")
                    for k in range(CPB):
                        c = c0 + k
                        nc.tensor.matmul(ops_[:, k, :],
                                         eb[:, 128 * c:128 * (c + 1)], cT[:],
                                         start=True, stop=True)
                    ot = osb.tile([128, CPB, GT], F16, tag="ot")
                    nc.vector.tensor_tensor(ot[:], ops_[:],
                                            axt[:, c0:c0 + CPB, :], op=AL.add)
                    nc.sync.dma_start(
                        out_d[:, (g * HCH + c0) * GT:(g * HCH + c0 + CPB) * GT],
                        ot[:].rearrange("p k i -> p (k i)"))

    nc.compile()
    return nc


_NC_CACHE = None


def _get_nc():
    global _NC_CACHE
    if _NC_CACHE is None:
        _NC_CACHE = _build()
    return _NC_CACHE


def _prep_inputs(hidden_states, router_weight, router_bias, expert_bias):
    flat = np.ascontiguousarray(hidden_states.reshape(T, H), dtype=np.float32)
    rwt = np.ascontiguousarray(
        router_weight.T.reshape(HCH, 128, E).transpose(1, 0, 2).reshape(128, HCH * E)
    ).astype(np.float16)
    rb = np.ascontiguousarray(router_bias.reshape(E, 1)).astype(np.float32)
    eb = np.ascontiguousarray(expert_bias).astype(np.float16)
    eye = np.eye(128, dtype=np.float32)
    eye_h = eye.astype(np.float16)
    ones = np.ones((E, 128), dtype=np.float16)
    in_maps = []
    for cc in range(N_CORES):
        xc = flat[cc * T_PC:(cc + 1) * T_PC]              # [2048t, 2048h]
        xcT = np.ascontiguousarray(xc.T).astype(np.float16)   # [2048h, 2048t]
        # [h, t] -> [p, g, c, i]: h = 128c + p, t = 512g + i
        xt = np.ascontiguousarray(
            xcT.reshape(HCH, 128, NG, GT).transpose(1, 2, 0, 3)
        ).reshape(128, NG * HCH * GT)
        in_maps.append({
            "xt": xt,
            "rwt": rwt,
            "eb": eb,
            "rb": rb,
            "idf": eye,
            "idh": eye_h,
            "ones": ones,
        })
    return in_maps


def kernel(hidden_states, router_weight, router_bias, expert_bias):
    hidden_states = np.asarray(hidden_states, dtype=np.float32)
    router_weight = np.asarray(router_weight, dtype=np.float32)
    router_bias = np.asarray(router_bias, dtype=np.float32)
    expert_bias = np.asarray(expert_bias, dtype=np.float32)
    assert hidden_states.shape == (B, S, H)

    nc = _get_nc()
    in_maps = _prep_inputs(hidden_states, router_weight, router_bias, expert_bias)
    res = run_bass_kernel_spmd(nc, in_maps, list(range(N_CORES)))
    out = np.empty((T, H), dtype=np.float32)
    for cc in range(N_CORES):
        arr = np.asarray(res.results[cc]["out"]).reshape(128, NG, HCH, GT)
        # [p, g, c, i] -> [t, h]
        out[cc * T_PC:(cc + 1) * T_PC] = (
            arr.transpose(1, 3, 2, 0).reshape(T_PC, H).astype(np.float32))
    return out.reshape(B, S, H)


if __name__ == "__main__":
    rng = np.random.default_rng(0)
    hs = rng.standard_normal((B, S, H), dtype=np.float32)
    rw = rng.standard_normal((E, H), dtype=np.float32)
    rbv = np.zeros((E,), dtype=np.float32)
    ebv = (rng.standard_normal((E, H), dtype=np.float32) * 0.1).astype(np.float32)
    o = kernel(hidden_states=hs, router_weight=rw, router_bias=rbv, expert_bias=ebv)
    print("kernel out", o.shape, o.dtype, float(np.abs(o).mean()))
